# revision 16
# baseline (speedup 1.0000x reference)
"""GAT (3-layer graph attention + final linear) Trainium2 Bass kernel.

Problem: B=4 graphs, N=2048 atoms, D=128, H=256.
  per layer: h = relu(x @ W.T + b); e_ij = leaky_relu(f1_i + f2_j, 0.01)
  masked by adj; att = softmax_j(e); x = x + att @ h.
  final: relu(x @ Wt.T + bt).

Sharding: 8 cores; core c -> (graph b=c//2, row-half s=c%2 of the NxN
attention). Per-core the atom (j) axis is reordered to [own half |
other half]; the between-layer exchange is an AllReduce(add) over the
pair (other = sum - mine), overlapped with the own-half j-tiles.

Engine budget per layer-per-core (the design target):
  exp separability: exp(f1_i+f2_j) = exp(f1_i)*exp(f2_j), so per j-tile
  [128 x 1024] only: q1 = T1*E2_j (DVE tensor_scalar, 4x bf16 mode),
  p = (T1s*e2_j) max q1 (Pool scalar_tensor_tensor), p *= adjT_j
  (DVE tensor_tensor; bf16 2x). PE: psAT += hnat_j @ p (bf16) and
  psS_row += ones_col @ p ([1,NS] row sum). All stationaries bf16.
  1/S via one reciprocal_approx_fast on the [1,NS] row + K=1 PE
  broadcast. h/relu+bias on ACT straight to bf16; f1&f2 in one
  2-column matmul. Adjacency arrives pre-transposed bf16 from host.
"""

import numpy as np
try:
    from ml_dtypes import bfloat16 as _BF16NP
except ImportError:
    import jax.numpy as _jnp
    _BF16NP = _jnp.bfloat16

import concourse.bass as bass
import concourse.mybir as mybir
import concourse.tile as tile
F32R = mybir.dt.float32r
from concourse import masks
from concourse.bass_utils import run_bass_kernel_spmd

P = 128
F32 = mybir.dt.float32
BF16 = mybir.dt.bfloat16
AF = mybir.ActivationFunctionType
OP = mybir.AluOpType


def _legalize_waits(nc, dma_limit=1, engine_limit=1):
    """Walrus can encode only 1 sem wait on a DMA instruction, 0 on an
    XBAR-transpose DMA, and ~2 on an engine instruction. Move excess
    waits onto standalone EventSemaphore instructions (1 wait each)
    inserted just before the offender on the same engine."""
    counter = [0]

    def split(ins):
        si = ins.sync_info
        if si is None:
            return None
        tn = type(ins).__name__
        if tn == "InstDmaTransposeAnt":
            limit = 0
        elif tn.startswith("InstDMA"):
            limit = dma_limit
        else:
            limit = engine_limit
        waits = list(si.on_wait)
        if len(waits) <= limit:
            return None
        keep = waits[-limit:] if limit > 0 else []
        extra = waits[:-limit] if limit > 0 else waits
        evs = []
        for w in extra:
            counter[0] += 1
            evs.append(mybir.InstEventSemaphore(
                name=f"evsplit{counter[0]}", engine=ins.engine,
                sync_info=mybir.SyncInfo(on_wait=[w], on_update=[])))
        ins.sync_info = mybir.SyncInfo(on_wait=keep,
                                       on_update=list(si.on_update))
        return evs

    for f in nc.m.functions:
        for blk in f.blocks:
            new_list = []
            changed = False
            for ins in blk.instructions:
                evs = split(ins)
                if evs:
                    new_list.extend(evs)
                    changed = True
                new_list.append(ins)
            if changed:
                blk.instructions = new_list


def build_gat_nc(N, NS, D, H, num_cores, pair_groups, nlayers=3,
                 legalize=True):
    assert D == P and NS % 512 == 0 and N % 512 == 0
    nj = N // P        # j tiles (core-local atom order: 0..7 own, 8..15 other)
    njh = nj // 2
    nch = NS // 512    # 512-chunks in shard
    nH = H // P

    nc = bass.Bass("TRN2", target_bir_lowering=False, debug=False,
                   num_devices=num_cores)

    # ---- I/O ----
    xTb_in = nc.dram_tensor("xTb", [P, N], BF16, kind="ExternalInput")
    adjT_in = nc.dram_tensor("adjT_s", [N, NS], BF16, kind="ExternalInput")
    WT_in = [nc.dram_tensor(f"WT{l}", [D, D], BF16, kind="ExternalInput")
             for l in range(nlayers)]
    bv_in = [nc.dram_tensor(f"bv{l}", [D, 1], F32, kind="ExternalInput")
             for l in range(nlayers)]
    av_in = [nc.dram_tensor(f"av{l}", [D, 2], BF16, kind="ExternalInput")
             for l in range(nlayers)]
    WtT_in = nc.dram_tensor("WtT", [D, H], BF16, kind="ExternalInput")
    btp_in = nc.dram_tensor("btp", [P, nH], F32, kind="ExternalInput")
    out_ext = nc.dram_tensor("outT_s", [H, NS], BF16, kind="ExternalOutput")

    # DRAM bounce buffers for the pair AllReduce of x shards (bf16)
    ar_in = [nc.dram_tensor(f"ar_in{l}", [P, NS], BF16)
             for l in range(nlayers - 1)]
    ar_out = [nc.dram_tensor(f"ar_out{l}", [P, NS], BF16)
              for l in range(nlayers - 1)]
    # warmup/startup collective (absorbs CC-stream init latency)
    arw_in = nc.dram_tensor("arw_in", [1, 1], F32)
    arw_out = nc.dram_tensor("arw_out", [1, 1], F32)

    with tile.TileContext(nc) as tc:
        import contextlib
        ctx = contextlib.ExitStack()
        with ctx:
            persist = ctx.enter_context(tc.tile_pool(name="persist", bufs=1))
            xtp = ctx.enter_context(tc.tile_pool(name="xtp", bufs=2))
            qp = ctx.enter_context(tc.tile_pool(name="qp", bufs=3))
            ocp = ctx.enter_context(tc.tile_pool(name="ocp", bufs=2))
            hp = ctx.enter_context(
                tc.tile_pool(name="hp", bufs=2, space="PSUM"))
            bigp = ctx.enter_context(
                tc.tile_pool(name="bigp", bufs=1, space="PSUM"))
            attp = ctx.enter_context(
                tc.tile_pool(name="attp", bufs=1, space="PSUM"))
            srp = ctx.enter_context(
                tc.tile_pool(name="srp", bufs=1, space="PSUM"))

            onesrow = persist.tile([1, P], F32)
            nc.vector.memset(onesrow[:], 1.0)
            onesrow_r = persist.tile([1, P], F32R)
            nc.vector.tensor_copy(onesrow_r[:], onesrow[:])
            onescol = persist.tile([P, 1], BF16)
            nc.vector.memset(onescol[:], 1.0)
            identb = persist.tile([P, P], BF16)
            masks.make_identity(nc, identb[:])

            # ---- persistent state ----
            adjT = persist.tile([P, nj, NS], BF16)
            p_all = persist.tile([P, nj, NS], BF16)
            hTb = persist.tile([P, N], BF16)
            hnat = persist.tile([P, nj, P], BF16)
            f1row = persist.tile([1, NS], F32R)
            E2c = persist.tile([P, nj], F32)
            e2c = persist.tile([P, nj], F32)
            T1 = persist.tile([P, NS], BF16)
            T1s = persist.tile([P, NS], BF16)
            Rv_row = persist.tile([1, NS], F32)
            rrow = persist.tile([1, NS], F32R)
            Rvb = persist.tile([P, NS], BF16)
            sumb = [persist.tile([P, NS], BF16, name=f"sumb{l}",
                                 tag=f"sumb{l}") for l in range(nlayers - 1)]
            xob = [persist.tile([P, NS], BF16, name=f"xob{l}",
                                tag=f"xob{l}") for l in range(nlayers - 1)]

            # raw DMA'd weights + laundered copies (so matmuls never
            # depend on two DMA queues)
            WT_d = [persist.tile([D, D], BF16, name=f"WTd{l}", tag=f"WTd{l}")
                    for l in range(nlayers)]
            bv_d = [persist.tile([D, 1], F32, name=f"bvd{l}", tag=f"bvd{l}")
                    for l in range(nlayers)]
            av_d = [persist.tile([D, 2], BF16, name=f"avd{l}", tag=f"avd{l}")
                    for l in range(nlayers)]
            WtT_d = persist.tile([D, H], BF16)
            btp_d = persist.tile([P, nH], F32)
            WT = [persist.tile([D, D], BF16, name=f"WTl{l}", tag=f"WTl{l}")
                  for l in range(nlayers)]
            bv = [persist.tile([D, 1], F32, name=f"bvl{l}", tag=f"bvl{l}")
                  for l in range(nlayers)]
            av = [persist.tile([D, 2], BF16, name=f"avl{l}", tag=f"avl{l}")
                  for l in range(nlayers)]
            WtTt = persist.tile([D, H], BF16)
            btpt = persist.tile([P, nH], F32)

            xTb = persist.tile([P, N], BF16)

            # ---- input DMAs: weights + x first (gate layer-0 PE), then
            # adjacency tiles in j order split across both HWDGE queues ----
            for l in range(nlayers):
                nc.sync.dma_start(WT_d[l][:], WT_in[l].ap())
                nc.sync.dma_start(bv_d[l][:], bv_in[l].ap())
                nc.sync.dma_start(av_d[l][:], av_in[l].ap())
                nc.vector.tensor_copy(WT[l][:], WT_d[l][:])
                nc.vector.tensor_copy(bv[l][:], bv_d[l][:])
                nc.vector.tensor_copy(av[l][:], av_d[l][:])
            nc.sync.dma_start(WtT_d[:], WtT_in.ap())
            nc.sync.dma_start(btp_d[:], btp_in.ap())
            nc.vector.tensor_copy(WtTt[:], WtT_d[:])
            nc.vector.tensor_copy(btpt[:], btp_d[:])
            for ch in range(2 * nch):
                csl = slice(ch * 512, (ch + 1) * 512)
                eng = nc.scalar if ch % 2 == 0 else nc.sync
                eng.dma_start(xTb[:, csl], xTb_in.ap()[:, csl])
            for j in range(nj):
                eng = nc.sync if j % 2 == 0 else nc.scalar
                eng.dma_start(adjT[:, j, :],
                              adjT_in.ap()[j * P:(j + 1) * P, :])

            # startup CC warmup (engines never wait on it; absorbs the
            # first-collective trigger latency seen in traces)
            nc.gpsimd.collective_compute(
                "AllReduce", OP.add, replica_groups=pair_groups,
                ins=[arw_in.ap()], outs=[arw_out.ap()])

            xb_cur = [None]  # [P, NS] bf16 view of current layer's own x

            def cur_x(sl):
                t, off = xb_cur[0]
                return t[:, off + sl.start:off + sl.stop]

            def do_half(l, half, src_of):
                """h (relu, bf16), f1 row (own half), hnat tiles, f2
                columns + E2c/e2c for one half of the atom axis.
                src_of(ch)->[P,512] bf16 view."""
                base = half * NS
                for ch in range(nch):
                    sl = slice(base + ch * 512, base + (ch + 1) * 512)
                    ps = hp.tile([P, 512], F32, name=f"hps{l}_{half}_{ch}",
                                 tag="hps")
                    nc.tensor.matmul(ps[:], WT[l][:], src_of(ch),
                                     start=True, stop=True)
                    nc.scalar.activation(hTb[:, sl], ps[:], AF.Relu,
                                         bias=bv[l][:])
                    if half == 0:
                        psf = hp.tile([1, 512], F32,
                                      name=f"fps{l}_{ch}", tag="hps")
                        nc.tensor.matmul(psf[:], av[l][:, 0:1], hTb[:, sl],
                                         start=True, stop=True)
                        nc.scalar.activation(
                            f1row[:, ch * 512:(ch + 1) * 512], psf[:],
                            AF.Copy)
                # h natural tiles via PE transposes (4-wide groups)
                for g in range(njh // 4):
                    pst = hp.tile([P, 512], BF16,
                                  name=f"htp{l}_{half}_{g}", tag="hps")
                    for q in range(4):
                        j = half * njh + g * 4 + q
                        nc.tensor.transpose(pst[:, q * P:(q + 1) * P],
                                            hTb[:, j * P:(j + 1) * P],
                                            identb[:])
                    nc.vector.tensor_copy(
                        hnat[:, half * njh + g * 4:half * njh + g * 4 + 4, :],
                        pst[:])
                # f2 columns directly: per-tile 1-col matmul with the hTb
                # tile as stationary, then exp / exp(0.01 .) out of PSUM
                psf2 = hp.tile([P, njh], F32, name=f"f2ps{l}_{half}",
                               tag="hps")
                for q in range(njh):
                    j = half * njh + q
                    nc.tensor.matmul(psf2[:, q:q + 1],
                                     hTb[:, j * P:(j + 1) * P],
                                     av[l][:, 1:2], start=True, stop=True)
                jsl = slice(half * njh, (half + 1) * njh)
                nc.scalar.activation(E2c[:, jsl], psf2[:], AF.Exp)
                nc.scalar.activation(e2c[:, jsl], psf2[:], AF.Exp,
                                     scale=0.01)

            def do_jtiles(l, psAT, psS, j0, j1):
                for j in range(j0, j1):
                    pj = p_all[:, j, :]
                    # q1 = T1 * E2_j  (DVE tensor_scalar, 2x/4x bf16)
                    nc.vector.tensor_scalar(pj, T1[:], E2c[:, j:j + 1],
                                            None, OP.mult)
                    # q2 = T1s * e2_j  (ACT/Pool alternating; Pool and ACT
                    # cannot do tensor-tensor max, so max stays on DVE)
                    q2 = qp.tile([P, NS], BF16, name=f"q2_{l}_{j}",
                                 tag="q2")
                    if j % 3 == 2:
                        nc.gpsimd.tensor_scalar(q2[:], T1s[:],
                                                e2c[:, j:j + 1], None,
                                                OP.mult)
                    else:
                        nc.scalar.activation(q2[:], T1s[:], AF.Copy,
                                             scale=e2c[:, j:j + 1])
                    nc.vector.tensor_tensor(pj, pj, q2[:], OP.max)
                    # mask: p *= adjT_j (Pool: mult is supported there)
                    nc.gpsimd.tensor_tensor(pj, pj, adjT[:, j, :], OP.mult)
                    for ch in range(nch):
                        sl = slice(ch * 512, (ch + 1) * 512)
                        nc.tensor.matmul(psAT[:, sl], hnat[:, j, :],
                                         p_all[:, j, sl],
                                         start=(j == 0), stop=(j == nj - 1))
                    for ch in range(nch):
                        sl = slice(ch * 512, (ch + 1) * 512)
                        nc.tensor.matmul(psS[0:1, sl], onescol[:],
                                         p_all[:, j, sl],
                                         start=(j == 0), stop=(j == nj - 1))

            for l in range(nlayers):
                last = l == nlayers - 1
                if l == 0:
                    xb_cur[0] = (xTb, 0)

                    def src0(ch, _l=l):
                        return xTb[:, ch * 512:(ch + 1) * 512]

                    def src1(ch, _l=l):
                        return xTb[:, NS + ch * 512:NS + (ch + 1) * 512]
                else:
                    def src0(ch, _l=l):
                        return cur_x(slice(ch * 512, (ch + 1) * 512))

                    def src1(ch, _l=l):
                        return xob[_l - 1][:, ch * 512:(ch + 1) * 512]

                # own half (no collective dependency)
                do_half(l, 0, src0)
                # T1 / T1s from f1 own rows: PE ones-outer bcast + ACT exp
                psF1 = bigp.tile([P, NS], F32, name=f"f1b{l}", tag="big")
                for ch in range(nch):
                    sl = slice(ch * 512, (ch + 1) * 512)
                    nc.tensor.matmul(psF1[:, sl], onesrow_r[:],
                                     f1row[0:1, sl], start=True, stop=True)
                nc.scalar.activation(T1[:], psF1[:], AF.Exp)
                nc.scalar.activation(T1s[:], psF1[:], AF.Exp, scale=0.01)

                psAT = attp.tile([P, NS], F32, name=f"psAT{l}", tag="att")
                psS = srp.tile([1, NS], F32, name=f"psS{l}", tag="srow")
                if l == 0:
                    do_half(l, 1, src1)
                    do_jtiles(l, psAT, psS, 0, nj)
                else:
                    # hide the AllReduce behind the own-half j-tiles
                    do_jtiles(l, psAT, psS, 0, njh)
                    nc.sync.dma_start(sumb[l - 1][:], ar_out[l - 1].ap())
                    nc.vector.tensor_tensor(xob[l - 1][:], sumb[l - 1][:],
                                            cur_x(slice(0, NS)),
                                            OP.subtract)
                    do_half(l, 1, src1)
                    do_jtiles(l, psAT, psS, njh, nj)

                # normalize + residual: x_new = psAT/S + x, chunked so the
                # next layer's h starts on chunk 0 early.
                # 1/S = exp(-ln(S)) on ACT (custom-DVE recip doesn't lower
                # on this toolchain; InstReciprocal costs ~6x elem rate)
                nc.scalar.activation(Rv_row[:], psS[:], AF.Ln)
                nc.scalar.activation(rrow[:], Rv_row[:], AF.Exp,
                                     scale=-1.0)
                psRv = bigp.tile([P, NS], F32, name=f"rvb{l}", tag="big")
                xb_new = xtp.tile([P, NS], BF16, name=f"xb{l + 1}",
                                  tag="xb")
                tmpb = qp.tile([P, NS], BF16, name=f"tmpb{l}", tag="q2")
                for ch in range(nch):
                    sl = slice(ch * 512, (ch + 1) * 512)
                    nc.tensor.matmul(psRv[:, sl], onesrow_r[:],
                                     rrow[0:1, sl], start=True, stop=True)
                    nc.scalar.activation(Rvb[:, sl], psRv[:, sl], AF.Copy)
                    nc.vector.tensor_tensor(tmpb[:, sl], psAT[:, sl],
                                            Rvb[:, sl], OP.mult)
                    nc.gpsimd.tensor_tensor(xb_new[:, sl], tmpb[:, sl],
                                            cur_x(sl), OP.add)
                xb_cur[0] = (xb_new, 0)

                if not last:
                    nc.gpsimd.dma_start(ar_in[l].ap(), xb_new[:])
                    nc.gpsimd.collective_compute(
                        "AllReduce", OP.add, replica_groups=pair_groups,
                        ins=[ar_in[l].ap()], outs=[ar_out[l].ap()])

            # ---- final linear: outT = relu(Wt @ x + bt), bf16, transposed
            for g in range(nH):
                for ch in range(nch):
                    sl = slice(ch * 512, (ch + 1) * 512)
                    ps = hp.tile([P, 512], F32, name=f"ops{g}_{ch}",
                                 tag="hps")
                    nc.tensor.matmul(ps[:], WtTt[:, g * P:(g + 1) * P],
                                     cur_x(sl), start=True, stop=True)
                    oc = ocp.tile([P, 512], BF16, name=f"oc{g}_{ch}",
                                  tag="oc")
                    nc.scalar.activation(oc[:], ps[:], AF.Relu,
                                         bias=btpt[:, g:g + 1])
                    nc.sync.dma_start(
                        out_ext.ap()[g * P:(g + 1) * P, sl], oc[:])

    if legalize:
        _legalize_waits(nc)
    return nc


def make_in_maps(x, adj, Ws, bs, avs, Wt, bt, num_cores, NS):
    """Per-core input dicts. Core c -> (graph c//2, row-half c%2).
    Per-core the atom (column) axis is permuted to [own half | other];
    adjacency is sent pre-transposed ([j, i]) in bf16."""
    B, N, D = x.shape
    H = Wt.shape[0]
    nH = H // P
    x = np.asarray(x, np.float32)
    adj = np.asarray(adj)
    shared = {"WtT": np.ascontiguousarray(
                  np.asarray(Wt, np.float32).T.astype(_BF16NP)),
              "btp": np.ascontiguousarray(
                  np.asarray(bt, np.float32).reshape(nH, P).T),
              "arw_in": np.zeros((1, 1), np.float32)}
    for l, (W, b, a) in enumerate(zip(Ws, bs, avs)):
        shared[f"WT{l}"] = np.ascontiguousarray(
            np.asarray(W, np.float32).T.astype(_BF16NP))
        shared[f"bv{l}"] = np.ascontiguousarray(
            np.asarray(b, np.float32).reshape(D, 1))
        shared[f"av{l}"] = np.ascontiguousarray(
            np.stack([np.asarray(a, np.float32)[:D, 0],
                      np.asarray(a, np.float32)[D:, 0]],
                     axis=1).astype(_BF16NP))
    in_maps = []
    for c in range(num_cores):
        b, s = c // 2, c % 2
        perm = np.concatenate([np.arange(s * NS, (s + 1) * NS),
                               np.arange((1 - s) * NS, (2 - s) * NS)])
        m = dict(shared)
        m["xTb"] = np.ascontiguousarray(x[b][perm].T.astype(_BF16NP))
        m["adjT_s"] = np.ascontiguousarray(
            adj[b, s * NS:(s + 1) * NS][:, perm].T.astype(_BF16NP))
        in_maps.append(m)
    return in_maps


_NC_CACHE = {}


def kernel(x, adj, W0, b0, W1, b1, W2, b2, a0, a1, a2, Wt, bt):
    B, N, D = 4, 2048, 128
    H = 256
    NUM_CORES = 8
    NS = N // 2
    pair_groups = [[2 * i, 2 * i + 1] for i in range(NUM_CORES // 2)]

    key = (N, NS, D, H, NUM_CORES)
    if key not in _NC_CACHE:
        _NC_CACHE[key] = build_gat_nc(N, NS, D, H, NUM_CORES, pair_groups)
    nc = _NC_CACHE[key]

    in_maps = make_in_maps(np.asarray(x), np.asarray(adj),
                           [W0, W1, W2], [b0, b1, b2], [a0, a1, a2],
                           np.asarray(Wt), np.asarray(bt), NUM_CORES, NS)
    res = run_bass_kernel_spmd(nc, in_maps, list(range(NUM_CORES))).results
    out = np.empty((B, N, H), np.float32)
    for c in range(NUM_CORES):
        b, s = c // 2, c % 2
        out[b, s * NS:(s + 1) * NS, :] = \
            res[c]["outT_s"].astype(np.float32).T
    return out


# revision 18
# speedup vs baseline: 2.6463x; 2.6463x over previous
"""GAT (3-layer graph attention + final linear) Trainium2 Bass kernel.

Problem: B=4 graphs, N=2048 atoms, D=128, H=256.
  per layer: h = relu(x @ W.T + b); e_ij = leaky_relu(f1_i + f2_j, 0.01)
  masked by adj; att = softmax_j(e); x = x + att @ h.
  final: relu(x @ Wt.T + bt).

Sharding: 8 cores; core c -> (graph b=c//2, row-half s=c%2 of the NxN
attention). Per-core the atom (j) axis is reordered to [own half |
other half]; the between-layer exchange is an AllReduce(add) over the
pair (other = sum - mine), overlapped with the own-half j-tiles.

Engine budget per layer-per-core (the design target):
  exp separability: exp(f1_i+f2_j) = exp(f1_i)*exp(f2_j), so per j-tile
  [128 x 1024] only: q1 = T1*E2_j (DVE tensor_scalar, 4x bf16 mode),
  p = (T1s*e2_j) max q1 (Pool scalar_tensor_tensor), p *= adjT_j
  (DVE tensor_tensor; bf16 2x). PE: psAT += hnat_j @ p (bf16) and
  psS_row += ones_col @ p ([1,NS] row sum). All stationaries bf16.
  1/S via one reciprocal_approx_fast on the [1,NS] row + K=1 PE
  broadcast. h/relu+bias on ACT straight to bf16; f1&f2 in one
  2-column matmul. Adjacency arrives pre-transposed bf16 from host.
"""

import numpy as np
try:
    from ml_dtypes import bfloat16 as _BF16NP
except ImportError:
    import jax.numpy as _jnp
    _BF16NP = _jnp.bfloat16

import concourse.bass as bass
import concourse.mybir as mybir
import concourse.tile as tile
F32R = mybir.dt.float32r
from concourse import masks
from concourse.bass_utils import run_bass_kernel_spmd

P = 128
F32 = mybir.dt.float32
BF16 = mybir.dt.bfloat16
AF = mybir.ActivationFunctionType
OP = mybir.AluOpType


def _legalize_waits(nc, dma_limit=1, engine_limit=1):
    """Walrus can encode only 1 sem wait on a DMA instruction, 0 on an
    XBAR-transpose DMA, and ~2 on an engine instruction. Move excess
    waits onto standalone EventSemaphore instructions (1 wait each)
    inserted just before the offender on the same engine."""
    counter = [0]

    def split(ins):
        si = ins.sync_info
        if si is None:
            return None
        tn = type(ins).__name__
        if tn == "InstDmaTransposeAnt":
            limit = 0
        elif tn.startswith("InstDMA"):
            limit = dma_limit
        else:
            limit = engine_limit
        waits = list(si.on_wait)
        if len(waits) <= limit:
            return None
        keep = waits[-limit:] if limit > 0 else []
        extra = waits[:-limit] if limit > 0 else waits
        evs = []
        for w in extra:
            counter[0] += 1
            evs.append(mybir.InstEventSemaphore(
                name=f"evsplit{counter[0]}", engine=ins.engine,
                sync_info=mybir.SyncInfo(on_wait=[w], on_update=[])))
        ins.sync_info = mybir.SyncInfo(on_wait=keep,
                                       on_update=list(si.on_update))
        return evs

    for f in nc.m.functions:
        for blk in f.blocks:
            new_list = []
            changed = False
            for ins in blk.instructions:
                evs = split(ins)
                if evs:
                    new_list.extend(evs)
                    changed = True
                new_list.append(ins)
            if changed:
                blk.instructions = new_list


def build_gat_nc(N, NS, D, H, num_cores, pair_groups, nlayers=3,
                 legalize=True):
    assert D == P and NS % 512 == 0 and N % 512 == 0
    nj = N // P        # j tiles (core-local atom order: 0..7 own, 8..15 other)
    njh = nj // 2
    nch = NS // 512    # 512-chunks in shard
    nH = H // P

    nc = bass.Bass("TRN2", target_bir_lowering=False, debug=False,
                   num_devices=num_cores)

    # ---- I/O ----
    xTb_in = nc.dram_tensor("xTb", [P, N], BF16, kind="ExternalInput")
    adjT_in = nc.dram_tensor("adjT_s", [N, NS], BF16, kind="ExternalInput")
    WT_in = [nc.dram_tensor(f"WT{l}", [D, D], BF16, kind="ExternalInput")
             for l in range(nlayers)]
    bv_in = [nc.dram_tensor(f"bv{l}", [D, 1], F32, kind="ExternalInput")
             for l in range(nlayers)]
    av_in = [nc.dram_tensor(f"av{l}", [D, 2], BF16, kind="ExternalInput")
             for l in range(nlayers)]
    WtT_in = nc.dram_tensor("WtT", [D, H], BF16, kind="ExternalInput")
    btp_in = nc.dram_tensor("btp", [P, nH], F32, kind="ExternalInput")
    out_ext = nc.dram_tensor("outT_s", [H, NS], BF16, kind="ExternalOutput")

    # DRAM bounce buffers for the pair AllReduce of x shards (bf16)
    ar_in = [nc.dram_tensor(f"ar_in{l}", [P, NS], BF16)
             for l in range(nlayers - 1)]
    ar_out = [nc.dram_tensor(f"ar_out{l}", [P, NS], BF16)
              for l in range(nlayers - 1)]
    # warmup/startup collective (absorbs CC-stream init latency)
    arw_in = nc.dram_tensor("arw_in", [1, 1], F32)
    arw_out = nc.dram_tensor("arw_out", [1, 1], F32)

    with tile.TileContext(nc) as tc:
        import contextlib
        ctx = contextlib.ExitStack()
        with ctx:
            persist = ctx.enter_context(tc.tile_pool(name="persist", bufs=1))
            xtp = ctx.enter_context(tc.tile_pool(name="xtp", bufs=2))
            qp = ctx.enter_context(tc.tile_pool(name="qp", bufs=3))
            ocp = ctx.enter_context(tc.tile_pool(name="ocp", bufs=2))
            hp = ctx.enter_context(
                tc.tile_pool(name="hp", bufs=2, space="PSUM"))
            bigp = ctx.enter_context(
                tc.tile_pool(name="bigp", bufs=1, space="PSUM"))
            attp = ctx.enter_context(
                tc.tile_pool(name="attp", bufs=1, space="PSUM"))
            srp = ctx.enter_context(
                tc.tile_pool(name="srp", bufs=1, space="PSUM"))

            onesrow = persist.tile([1, P], F32)
            nc.vector.memset(onesrow[:], 1.0)
            onesrow_r = persist.tile([1, P], F32R)
            nc.vector.tensor_copy(onesrow_r[:], onesrow[:])
            onescol = persist.tile([P, 1], BF16)
            nc.vector.memset(onescol[:], 1.0)
            identb = persist.tile([P, P], BF16)
            masks.make_identity(nc, identb[:])

            # ---- persistent state ----
            adjT = persist.tile([P, nj, NS], BF16)
            p_all = persist.tile([P, nj, NS], BF16)
            hTb = persist.tile([P, N], BF16)
            hnat = persist.tile([P, nj, P], BF16)
            f1row = persist.tile([1, NS], F32R)
            E2c = persist.tile([P, nj], F32)
            e2c = persist.tile([P, nj], F32)
            T1 = persist.tile([P, NS], BF16)
            T1s = persist.tile([P, NS], BF16)
            Rv_row = persist.tile([1, NS], F32)
            rrow = persist.tile([1, NS], F32R)
            Rvb = persist.tile([P, NS], BF16)
            sumb = [persist.tile([P, NS], BF16, name=f"sumb{l}",
                                 tag=f"sumb{l}") for l in range(nlayers - 1)]
            xob = [persist.tile([P, NS], BF16, name=f"xob{l}",
                                tag=f"xob{l}") for l in range(nlayers - 1)]

            # raw DMA'd weights + laundered copies (so matmuls never
            # depend on two DMA queues)
            WT_d = [persist.tile([D, D], BF16, name=f"WTd{l}", tag=f"WTd{l}")
                    for l in range(nlayers)]
            bv_d = [persist.tile([D, 1], F32, name=f"bvd{l}", tag=f"bvd{l}")
                    for l in range(nlayers)]
            av_d = [persist.tile([D, 2], BF16, name=f"avd{l}", tag=f"avd{l}")
                    for l in range(nlayers)]
            WtT_d = persist.tile([D, H], BF16)
            btp_d = persist.tile([P, nH], F32)
            WT = [persist.tile([D, D], BF16, name=f"WTl{l}", tag=f"WTl{l}")
                  for l in range(nlayers)]
            bv = [persist.tile([D, 1], F32, name=f"bvl{l}", tag=f"bvl{l}")
                  for l in range(nlayers)]
            av = [persist.tile([D, 2], BF16, name=f"avl{l}", tag=f"avl{l}")
                  for l in range(nlayers)]
            WtTt = persist.tile([D, H], BF16)
            btpt = persist.tile([P, nH], F32)

            xTb = persist.tile([P, N], BF16)

            # ---- input DMAs: weights + x first (gate layer-0 PE), then
            # adjacency tiles in j order split across both HWDGE queues ----
            for l in range(nlayers):
                nc.sync.dma_start(WT_d[l][:], WT_in[l].ap())
                nc.sync.dma_start(bv_d[l][:], bv_in[l].ap())
                nc.sync.dma_start(av_d[l][:], av_in[l].ap())
                nc.vector.tensor_copy(WT[l][:], WT_d[l][:])
                nc.vector.tensor_copy(bv[l][:], bv_d[l][:])
                nc.vector.tensor_copy(av[l][:], av_d[l][:])
            nc.sync.dma_start(WtT_d[:], WtT_in.ap())
            nc.sync.dma_start(btp_d[:], btp_in.ap())
            nc.vector.tensor_copy(WtTt[:], WtT_d[:])
            nc.vector.tensor_copy(btpt[:], btp_d[:])
            for ch in range(2 * nch):
                csl = slice(ch * 512, (ch + 1) * 512)
                eng = nc.scalar if ch % 2 == 0 else nc.sync
                eng.dma_start(xTb[:, csl], xTb_in.ap()[:, csl])
            for j in range(nj):
                eng = nc.sync if j % 2 == 0 else nc.scalar
                eng.dma_start(adjT[:, j, :],
                              adjT_in.ap()[j * P:(j + 1) * P, :])

            # startup CC warmup (engines never wait on it; absorbs the
            # first-collective trigger latency seen in traces)
            nc.gpsimd.collective_compute(
                "AllReduce", OP.add, replica_groups=pair_groups,
                ins=[arw_in.ap()], outs=[arw_out.ap()])

            xb_cur = [None]  # [P, NS] bf16 view of current layer's own x

            def cur_x(sl):
                t, off = xb_cur[0]
                return t[:, off + sl.start:off + sl.stop]

            def do_half(l, half, src_of):
                """h (relu, bf16), f1 row (own half), hnat tiles, f2
                columns + E2c/e2c for one half of the atom axis.
                src_of(ch)->[P,512] bf16 view."""
                base = half * NS
                for ch in range(nch):
                    sl = slice(base + ch * 512, base + (ch + 1) * 512)
                    ps = hp.tile([P, 512], F32, name=f"hps{l}_{half}_{ch}",
                                 tag="hps")
                    nc.tensor.matmul(ps[:], WT[l][:], src_of(ch),
                                     start=True, stop=True)
                    nc.scalar.activation(hTb[:, sl], ps[:], AF.Relu,
                                         bias=bv[l][:])
                    if half == 0:
                        psf = hp.tile([1, 512], F32,
                                      name=f"fps{l}_{ch}", tag="hps")
                        nc.tensor.matmul(psf[:], av[l][:, 0:1], hTb[:, sl],
                                         start=True, stop=True)
                        nc.scalar.activation(
                            f1row[:, ch * 512:(ch + 1) * 512], psf[:],
                            AF.Copy)
                # h natural tiles via PE transposes (4-wide groups)
                for g in range(njh // 4):
                    pst = hp.tile([P, 512], BF16,
                                  name=f"htp{l}_{half}_{g}", tag="hps")
                    for q in range(4):
                        j = half * njh + g * 4 + q
                        nc.tensor.transpose(pst[:, q * P:(q + 1) * P],
                                            hTb[:, j * P:(j + 1) * P],
                                            identb[:])
                    nc.vector.tensor_copy(
                        hnat[:, half * njh + g * 4:half * njh + g * 4 + 4, :],
                        pst[:])
                # f2 columns directly: per-tile 1-col matmul with the hTb
                # tile as stationary, then exp / exp(0.01 .) out of PSUM
                psf2 = hp.tile([P, njh], F32, name=f"f2ps{l}_{half}",
                               tag="hps")
                for q in range(njh):
                    j = half * njh + q
                    nc.tensor.matmul(psf2[:, q:q + 1],
                                     hTb[:, j * P:(j + 1) * P],
                                     av[l][:, 1:2], start=True, stop=True)
                jsl = slice(half * njh, (half + 1) * njh)
                nc.scalar.activation(E2c[:, jsl], psf2[:], AF.Exp)
                nc.scalar.activation(e2c[:, jsl], psf2[:], AF.Exp,
                                     scale=0.01)

            def do_jtiles(l, psAT, psS, j0, j1):
                # tile PAIRS: q1/q2 per tile (per-tile scalars), max and
                # mask batched over the contiguous [P, 2*NS] pair to
                # amortize DVE per-instruction overhead. GpSimd is NOT
                # used for elementwise (its ucode loops run ~2.4-15us per
                # op and poison DVE throughput via SBUF contention).
                assert (j1 - j0) % 2 == 0
                for j in range(j0, j1, 2):
                    q2 = qp.tile([P, 2, NS], BF16, name=f"q2_{l}_{j}",
                                 tag="q2")
                    for k in (0, 1):
                        # q1 = T1 * E2_j (DVE tensor_scalar, bf16 fast mode)
                        nc.vector.tensor_scalar(p_all[:, j + k, :], T1[:],
                                                E2c[:, j + k:j + k + 1],
                                                None, OP.mult)
                        # q2 = T1s * e2_j (ACT copy-with-scale)
                        nc.scalar.activation(q2[:, k, :], T1s[:], AF.Copy,
                                             scale=e2c[:, j + k:j + k + 1])
                    pj2 = p_all[:, j:j + 2, :]
                    nc.vector.tensor_tensor(pj2, pj2, q2[:], OP.max)
                    nc.vector.tensor_tensor(pj2, pj2, adjT[:, j:j + 2, :],
                                            OP.mult)
                    for jj in (j, j + 1):
                        for ch in range(nch):
                            sl = slice(ch * 512, (ch + 1) * 512)
                            nc.tensor.matmul(psAT[:, sl], hnat[:, jj, :],
                                             p_all[:, jj, sl],
                                             start=(jj == 0),
                                             stop=(jj == nj - 1))
                        for ch in range(nch):
                            sl = slice(ch * 512, (ch + 1) * 512)
                            nc.tensor.matmul(psS[0:1, sl], onescol[:],
                                             p_all[:, jj, sl],
                                             start=(jj == 0),
                                             stop=(jj == nj - 1))

            for l in range(nlayers):
                last = l == nlayers - 1
                if l == 0:
                    xb_cur[0] = (xTb, 0)

                    def src0(ch, _l=l):
                        return xTb[:, ch * 512:(ch + 1) * 512]

                    def src1(ch, _l=l):
                        return xTb[:, NS + ch * 512:NS + (ch + 1) * 512]
                else:
                    def src0(ch, _l=l):
                        return cur_x(slice(ch * 512, (ch + 1) * 512))

                    def src1(ch, _l=l):
                        return xob[_l - 1][:, ch * 512:(ch + 1) * 512]

                # own half (no collective dependency)
                do_half(l, 0, src0)
                # T1 / T1s from f1 own rows: PE ones-outer bcast + ACT exp
                psF1 = bigp.tile([P, NS], F32, name=f"f1b{l}", tag="big")
                for ch in range(nch):
                    sl = slice(ch * 512, (ch + 1) * 512)
                    nc.tensor.matmul(psF1[:, sl], onesrow_r[:],
                                     f1row[0:1, sl], start=True, stop=True)
                nc.scalar.activation(T1[:], psF1[:], AF.Exp)
                nc.scalar.activation(T1s[:], psF1[:], AF.Exp, scale=0.01)

                psAT = attp.tile([P, NS], F32, name=f"psAT{l}", tag="att")
                psS = srp.tile([1, NS], F32, name=f"psS{l}", tag="srow")
                if l == 0:
                    do_half(l, 1, src1)
                    do_jtiles(l, psAT, psS, 0, nj)
                else:
                    # hide the AllReduce behind the own-half j-tiles
                    do_jtiles(l, psAT, psS, 0, njh)
                    nc.sync.dma_start(sumb[l - 1][:], ar_out[l - 1].ap())
                    nc.vector.tensor_tensor(xob[l - 1][:], sumb[l - 1][:],
                                            cur_x(slice(0, NS)),
                                            OP.subtract)
                    do_half(l, 1, src1)
                    do_jtiles(l, psAT, psS, njh, nj)

                # normalize + residual: x_new = psAT/S + x, chunked so the
                # next layer's h starts on chunk 0 early.
                # 1/S = exp(-ln(S)) on ACT (custom-DVE recip doesn't lower
                # on this toolchain; InstReciprocal costs ~6x elem rate)
                nc.scalar.activation(Rv_row[:], psS[:], AF.Ln)
                nc.scalar.activation(rrow[:], Rv_row[:], AF.Exp,
                                     scale=-1.0)
                psRv = bigp.tile([P, NS], F32, name=f"rvb{l}", tag="big")
                xb_new = xtp.tile([P, NS], BF16, name=f"xb{l + 1}",
                                  tag="xb")
                tmpb = qp.tile([P, NS], BF16, name=f"tmpb{l}", tag="q2")
                for ch in range(nch):
                    sl = slice(ch * 512, (ch + 1) * 512)
                    nc.tensor.matmul(psRv[:, sl], onesrow_r[:],
                                     rrow[0:1, sl], start=True, stop=True)
                    nc.scalar.activation(Rvb[:, sl], psRv[:, sl], AF.Copy)
                    nc.vector.tensor_tensor(tmpb[:, sl], psAT[:, sl],
                                            Rvb[:, sl], OP.mult)
                    nc.vector.tensor_tensor(xb_new[:, sl], tmpb[:, sl],
                                            cur_x(sl), OP.add)
                xb_cur[0] = (xb_new, 0)

                if not last:
                    nc.gpsimd.dma_start(ar_in[l].ap(), xb_new[:])
                    nc.gpsimd.collective_compute(
                        "AllReduce", OP.add, replica_groups=pair_groups,
                        ins=[ar_in[l].ap()], outs=[ar_out[l].ap()])

            # ---- final linear: outT = relu(Wt @ x + bt), bf16, transposed
            for g in range(nH):
                for ch in range(nch):
                    sl = slice(ch * 512, (ch + 1) * 512)
                    ps = hp.tile([P, 512], F32, name=f"ops{g}_{ch}",
                                 tag="hps")
                    nc.tensor.matmul(ps[:], WtTt[:, g * P:(g + 1) * P],
                                     cur_x(sl), start=True, stop=True)
                    oc = ocp.tile([P, 512], BF16, name=f"oc{g}_{ch}",
                                  tag="oc")
                    nc.scalar.activation(oc[:], ps[:], AF.Relu,
                                         bias=btpt[:, g:g + 1])
                    nc.sync.dma_start(
                        out_ext.ap()[g * P:(g + 1) * P, sl], oc[:])

    if legalize:
        _legalize_waits(nc)
    return nc


def make_in_maps(x, adj, Ws, bs, avs, Wt, bt, num_cores, NS):
    """Per-core input dicts. Core c -> (graph c//2, row-half c%2).
    Per-core the atom (column) axis is permuted to [own half | other];
    adjacency is sent pre-transposed ([j, i]) in bf16."""
    B, N, D = x.shape
    H = Wt.shape[0]
    nH = H // P
    x = np.asarray(x, np.float32)
    adj = np.asarray(adj)
    shared = {"WtT": np.ascontiguousarray(
                  np.asarray(Wt, np.float32).T.astype(_BF16NP)),
              "btp": np.ascontiguousarray(
                  np.asarray(bt, np.float32).reshape(nH, P).T),
              "arw_in": np.zeros((1, 1), np.float32)}
    for l, (W, b, a) in enumerate(zip(Ws, bs, avs)):
        shared[f"WT{l}"] = np.ascontiguousarray(
            np.asarray(W, np.float32).T.astype(_BF16NP))
        shared[f"bv{l}"] = np.ascontiguousarray(
            np.asarray(b, np.float32).reshape(D, 1))
        shared[f"av{l}"] = np.ascontiguousarray(
            np.stack([np.asarray(a, np.float32)[:D, 0],
                      np.asarray(a, np.float32)[D:, 0]],
                     axis=1).astype(_BF16NP))
    in_maps = []
    for c in range(num_cores):
        b, s = c // 2, c % 2
        perm = np.concatenate([np.arange(s * NS, (s + 1) * NS),
                               np.arange((1 - s) * NS, (2 - s) * NS)])
        m = dict(shared)
        m["xTb"] = np.ascontiguousarray(x[b][perm].T.astype(_BF16NP))
        m["adjT_s"] = np.ascontiguousarray(
            adj[b, s * NS:(s + 1) * NS][:, perm].T.astype(_BF16NP))
        in_maps.append(m)
    return in_maps


_NC_CACHE = {}


def kernel(x, adj, W0, b0, W1, b1, W2, b2, a0, a1, a2, Wt, bt):
    B, N, D = 4, 2048, 128
    H = 256
    NUM_CORES = 8
    NS = N // 2
    pair_groups = [[2 * i, 2 * i + 1] for i in range(NUM_CORES // 2)]

    key = (N, NS, D, H, NUM_CORES)
    if key not in _NC_CACHE:
        _NC_CACHE[key] = build_gat_nc(N, NS, D, H, NUM_CORES, pair_groups)
    nc = _NC_CACHE[key]

    in_maps = make_in_maps(np.asarray(x), np.asarray(adj),
                           [W0, W1, W2], [b0, b1, b2], [a0, a1, a2],
                           np.asarray(Wt), np.asarray(bt), NUM_CORES, NS)
    res = run_bass_kernel_spmd(nc, in_maps, list(range(NUM_CORES))).results
    out = np.empty((B, N, H), np.float32)
    for c in range(NUM_CORES):
        b, s = c // 2, c % 2
        out[b, s * NS:(s + 1) * NS, :] = \
            res[c]["outT_s"].astype(np.float32).T
    return out


# revision 19
# speedup vs baseline: 2.7322x; 1.0324x over previous
"""GAT (3-layer graph attention + final linear) Trainium2 Bass kernel.

Problem: B=4 graphs, N=2048 atoms, D=128, H=256.
  per layer: h = relu(x @ W.T + b); e_ij = leaky_relu(f1_i + f2_j, 0.01)
  masked by adj; att = softmax_j(e); x = x + att @ h.
  final: relu(x @ Wt.T + bt).

Sharding: 8 cores; core c -> (graph b=c//2, row-half s=c%2 of the NxN
attention). Per-core the atom (j) axis is reordered to [own half |
other half]; the between-layer exchange is an AllReduce(add) over the
pair (other = sum - mine), overlapped with the own-half j-tiles.

Engine budget per layer-per-core (the design target):
  exp separability: exp(f1_i+f2_j) = exp(f1_i)*exp(f2_j), so per j-tile
  [128 x 1024] only: q1 = T1*E2_j (DVE tensor_scalar, 4x bf16 mode),
  p = (T1s*e2_j) max q1 (Pool scalar_tensor_tensor), p *= adjT_j
  (DVE tensor_tensor; bf16 2x). PE: psAT += hnat_j @ p (bf16) and
  psS_row += ones_col @ p ([1,NS] row sum). All stationaries bf16.
  1/S via one reciprocal_approx_fast on the [1,NS] row + K=1 PE
  broadcast. h/relu+bias on ACT straight to bf16; f1&f2 in one
  2-column matmul. Adjacency arrives pre-transposed bf16 from host.
"""

import numpy as np
try:
    from ml_dtypes import bfloat16 as _BF16NP
except ImportError:
    import jax.numpy as _jnp
    _BF16NP = _jnp.bfloat16

import concourse.bass as bass
import concourse.mybir as mybir
import concourse.tile as tile
F32R = mybir.dt.float32r
from concourse import masks
from concourse.bass_utils import run_bass_kernel_spmd

P = 128
F32 = mybir.dt.float32
BF16 = mybir.dt.bfloat16
AF = mybir.ActivationFunctionType
OP = mybir.AluOpType


def _legalize_waits(nc, dma_limit=1, engine_limit=1):
    """Walrus can encode only 1 sem wait on a DMA instruction, 0 on an
    XBAR-transpose DMA, and ~2 on an engine instruction. Move excess
    waits onto standalone EventSemaphore instructions (1 wait each)
    inserted just before the offender on the same engine."""
    counter = [0]

    def split(ins):
        si = ins.sync_info
        if si is None:
            return None
        tn = type(ins).__name__
        if tn == "InstDmaTransposeAnt":
            limit = 0
        elif tn.startswith("InstDMA"):
            limit = dma_limit
        else:
            limit = engine_limit
        waits = list(si.on_wait)
        if len(waits) <= limit:
            return None
        keep = waits[-limit:] if limit > 0 else []
        extra = waits[:-limit] if limit > 0 else waits
        evs = []
        for w in extra:
            counter[0] += 1
            evs.append(mybir.InstEventSemaphore(
                name=f"evsplit{counter[0]}", engine=ins.engine,
                sync_info=mybir.SyncInfo(on_wait=[w], on_update=[])))
        ins.sync_info = mybir.SyncInfo(on_wait=keep,
                                       on_update=list(si.on_update))
        return evs

    for f in nc.m.functions:
        for blk in f.blocks:
            new_list = []
            changed = False
            for ins in blk.instructions:
                evs = split(ins)
                if evs:
                    new_list.extend(evs)
                    changed = True
                new_list.append(ins)
            if changed:
                blk.instructions = new_list


def build_gat_nc(N, NS, D, H, num_cores, pair_groups, nlayers=3,
                 legalize=True):
    assert D == P and NS % 512 == 0 and N % 512 == 0
    nj = N // P        # j tiles (core-local atom order: 0..7 own, 8..15 other)
    njh = nj // 2
    nch = NS // 512    # 512-chunks in shard
    nH = H // P

    nc = bass.Bass("TRN2", target_bir_lowering=False, debug=False,
                   num_devices=num_cores)

    # ---- I/O ----
    xTb_in = nc.dram_tensor("xTb", [P, N], BF16, kind="ExternalInput")
    adjT_in = nc.dram_tensor("adjT_s", [N, NS], BF16, kind="ExternalInput")
    WT_in = [nc.dram_tensor(f"WT{l}", [D, D], BF16, kind="ExternalInput")
             for l in range(nlayers)]
    bv_in = [nc.dram_tensor(f"bv{l}", [D, 1], F32, kind="ExternalInput")
             for l in range(nlayers)]
    av_in = [nc.dram_tensor(f"av{l}", [D, 2], BF16, kind="ExternalInput")
             for l in range(nlayers)]
    WtT_in = nc.dram_tensor("WtT", [D, H], BF16, kind="ExternalInput")
    btp_in = nc.dram_tensor("btp", [P, nH], F32, kind="ExternalInput")
    out_ext = nc.dram_tensor("outT_s", [H, NS], BF16, kind="ExternalOutput")

    # DRAM bounce buffers for the pair AllReduce of x shards (bf16)
    ar_in = [nc.dram_tensor(f"ar_in{l}", [P, NS], BF16)
             for l in range(nlayers - 1)]
    ar_out = [nc.dram_tensor(f"ar_out{l}", [P, NS], BF16)
              for l in range(nlayers - 1)]
    # warmup/startup collective (absorbs CC-stream init latency)
    arw_in = nc.dram_tensor("arw_in", [1, 1], F32)
    arw_out = nc.dram_tensor("arw_out", [1, 1], F32)

    with tile.TileContext(nc) as tc:
        import contextlib
        ctx = contextlib.ExitStack()
        with ctx:
            persist = ctx.enter_context(tc.tile_pool(name="persist", bufs=1))
            xtp = ctx.enter_context(tc.tile_pool(name="xtp", bufs=2))
            qp = ctx.enter_context(tc.tile_pool(name="qp", bufs=3))
            ocp = ctx.enter_context(tc.tile_pool(name="ocp", bufs=2))
            hp = ctx.enter_context(
                tc.tile_pool(name="hp", bufs=2, space="PSUM"))
            bigp = ctx.enter_context(
                tc.tile_pool(name="bigp", bufs=1, space="PSUM"))
            attp = ctx.enter_context(
                tc.tile_pool(name="attp", bufs=1, space="PSUM"))
            srp = ctx.enter_context(
                tc.tile_pool(name="srp", bufs=1, space="PSUM"))

            onesrow = persist.tile([1, P], F32)
            nc.vector.memset(onesrow[:], 1.0)
            onesrow_r = persist.tile([1, P], F32R)
            nc.vector.tensor_copy(onesrow_r[:], onesrow[:])
            onescol = persist.tile([P, 1], BF16)
            nc.vector.memset(onescol[:], 1.0)
            identb = persist.tile([P, P], BF16)
            masks.make_identity(nc, identb[:])

            # ---- persistent state ----
            adjT = persist.tile([P, nj, NS], BF16)
            p_all = persist.tile([P, nj, NS], BF16)
            hTb = persist.tile([P, N], BF16)
            hnat = persist.tile([P, nj, P], BF16)
            f1row = persist.tile([1, NS], F32R)
            rc = persist.tile([P, nj], F32)
            e2c = persist.tile([P, nj], F32)
            e2cb = persist.tile([P, nj], BF16)
            U = persist.tile([P, NS], BF16)
            Rv_row = persist.tile([1, NS], F32)
            rrow = persist.tile([1, NS], F32R)
            Rvb = persist.tile([P, NS], BF16)
            sumb = [persist.tile([P, NS], BF16, name=f"sumb{l}",
                                 tag=f"sumb{l}") for l in range(nlayers - 1)]
            xob = [persist.tile([P, NS], BF16, name=f"xob{l}",
                                tag=f"xob{l}") for l in range(nlayers - 1)]

            # raw DMA'd weights + laundered copies (so matmuls never
            # depend on two DMA queues)
            WT_d = [persist.tile([D, D], BF16, name=f"WTd{l}", tag=f"WTd{l}")
                    for l in range(nlayers)]
            bv_d = [persist.tile([D, 1], F32, name=f"bvd{l}", tag=f"bvd{l}")
                    for l in range(nlayers)]
            av_d = [persist.tile([D, 2], BF16, name=f"avd{l}", tag=f"avd{l}")
                    for l in range(nlayers)]
            WtT_d = persist.tile([D, H], BF16)
            btp_d = persist.tile([P, nH], F32)
            WT = [persist.tile([D, D], BF16, name=f"WTl{l}", tag=f"WTl{l}")
                  for l in range(nlayers)]
            bv = [persist.tile([D, 1], F32, name=f"bvl{l}", tag=f"bvl{l}")
                  for l in range(nlayers)]
            av = [persist.tile([D, 2], BF16, name=f"avl{l}", tag=f"avl{l}")
                  for l in range(nlayers)]
            WtTt = persist.tile([D, H], BF16)
            btpt = persist.tile([P, nH], F32)

            xTb = persist.tile([P, N], BF16)

            # ---- input DMAs: weights + x first (gate layer-0 PE), then
            # adjacency tiles in j order split across both HWDGE queues ----
            for l in range(nlayers):
                nc.sync.dma_start(WT_d[l][:], WT_in[l].ap())
                nc.sync.dma_start(bv_d[l][:], bv_in[l].ap())
                nc.sync.dma_start(av_d[l][:], av_in[l].ap())
                nc.vector.tensor_copy(WT[l][:], WT_d[l][:])
                nc.vector.tensor_copy(bv[l][:], bv_d[l][:])
                nc.vector.tensor_copy(av[l][:], av_d[l][:])
            nc.sync.dma_start(WtT_d[:], WtT_in.ap())
            nc.sync.dma_start(btp_d[:], btp_in.ap())
            nc.vector.tensor_copy(WtTt[:], WtT_d[:])
            nc.vector.tensor_copy(btpt[:], btp_d[:])
            for ch in range(2 * nch):
                csl = slice(ch * 512, (ch + 1) * 512)
                eng = nc.scalar if ch % 2 == 0 else nc.sync
                eng.dma_start(xTb[:, csl], xTb_in.ap()[:, csl])
            for j in range(nj):
                eng = nc.sync if j % 2 == 0 else nc.scalar
                eng.dma_start(adjT[:, j, :],
                              adjT_in.ap()[j * P:(j + 1) * P, :])

            # startup CC warmup (engines never wait on it; absorbs the
            # first-collective trigger latency seen in traces)
            nc.gpsimd.collective_compute(
                "AllReduce", OP.add, replica_groups=pair_groups,
                ins=[arw_in.ap()], outs=[arw_out.ap()])

            xb_cur = [None]  # [P, NS] bf16 view of current layer's own x

            def cur_x(sl):
                t, off = xb_cur[0]
                return t[:, off + sl.start:off + sl.stop]

            def do_half(l, half, src_of):
                """h (relu, bf16), f1 row (own half), hnat tiles, f2
                columns + E2c/e2c for one half of the atom axis.
                src_of(ch)->[P,512] bf16 view."""
                base = half * NS
                for ch in range(nch):
                    sl = slice(base + ch * 512, base + (ch + 1) * 512)
                    ps = hp.tile([P, 512], F32, name=f"hps{l}_{half}_{ch}",
                                 tag="hps")
                    nc.tensor.matmul(ps[:], WT[l][:], src_of(ch),
                                     start=True, stop=True)
                    nc.scalar.activation(hTb[:, sl], ps[:], AF.Relu,
                                         bias=bv[l][:])
                    if half == 0:
                        psf = hp.tile([1, 512], F32,
                                      name=f"fps{l}_{ch}", tag="hps")
                        nc.tensor.matmul(psf[:], av[l][:, 0:1], hTb[:, sl],
                                         start=True, stop=True)
                        nc.scalar.activation(
                            f1row[:, ch * 512:(ch + 1) * 512], psf[:],
                            AF.Copy)
                # f2 columns directly: per-tile 1-col matmul with the hTb
                # tile as stationary; then r = exp(.99 f2), e2 = exp(.01 f2)
                psf2 = hp.tile([P, njh], F32, name=f"f2ps{l}_{half}",
                               tag="hps")
                for q in range(njh):
                    j = half * njh + q
                    nc.tensor.matmul(psf2[:, q:q + 1],
                                     hTb[:, j * P:(j + 1) * P],
                                     av[l][:, 1:2], start=True, stop=True)
                jsl = slice(half * njh, (half + 1) * njh)
                nc.scalar.activation(rc[:, jsl], psf2[:], AF.Exp,
                                     scale=0.99)
                nc.scalar.activation(e2c[:, jsl], psf2[:], AF.Exp,
                                     scale=0.01)
                nc.vector.tensor_copy(e2cb[:, jsl], e2c[:, jsl])
                # h natural tiles via PE transposes (4-wide groups), scaled
                # per-partition by e2 on the PSUM->SBUF copy (ACT): the
                # e2_j attention factor rides the matmul stationaries.
                for g in range(njh // 4):
                    pst = hp.tile([P, 512], BF16,
                                  name=f"htp{l}_{half}_{g}", tag="hps")
                    for q in range(4):
                        j = half * njh + g * 4 + q
                        nc.tensor.transpose(pst[:, q * P:(q + 1) * P],
                                            hTb[:, j * P:(j + 1) * P],
                                            identb[:])
                    for q in range(4):
                        j = half * njh + g * 4 + q
                        nc.scalar.activation(hnat[:, j, :],
                                             pst[:, q * P:(q + 1) * P],
                                             AF.Copy,
                                             scale=e2c[:, j:j + 1])

            def do_jtiles(l, psAT, psS, j0, j1):
                # tile PAIRS: q1/q2 per tile (per-tile scalars), max and
                # mask batched over the contiguous [P, 2*NS] pair to
                # amortize DVE per-instruction overhead. GpSimd is NOT
                # used for elementwise (its ucode loops run ~2.4-15us per
                # op and poison DVE throughput via SBUF contention).
                assert (j1 - j0) % 2 == 0
                for j in range(j0, j1, 2):
                    for k in (0, 1):
                        # m = max(U * r_j, 1): ONE fused DVE tensor_scalar.
                        # The exp(.01 f1) row factor cancels in softmax;
                        # exp(.01 f2) rides the matmul stationaries.
                        nc.vector.tensor_scalar(p_all[:, j + k, :], U[:],
                                                rc[:, j + k:j + k + 1],
                                                1.0, OP.mult, OP.max)
                    pj2 = p_all[:, j:j + 2, :]
                    nc.vector.tensor_tensor(pj2, pj2, adjT[:, j:j + 2, :],
                                            OP.mult)
                    for jj in (j, j + 1):
                        for ch in range(nch):
                            sl = slice(ch * 512, (ch + 1) * 512)
                            nc.tensor.matmul(psAT[:, sl], hnat[:, jj, :],
                                             p_all[:, jj, sl],
                                             start=(jj == 0),
                                             stop=(jj == nj - 1))
                        for ch in range(nch):
                            sl = slice(ch * 512, (ch + 1) * 512)
                            nc.tensor.matmul(psS[0:1, sl],
                                             e2cb[:, jj:jj + 1],
                                             p_all[:, jj, sl],
                                             start=(jj == 0),
                                             stop=(jj == nj - 1))

            for l in range(nlayers):
                last = l == nlayers - 1
                if l == 0:
                    xb_cur[0] = (xTb, 0)

                    def src0(ch, _l=l):
                        return xTb[:, ch * 512:(ch + 1) * 512]

                    def src1(ch, _l=l):
                        return xTb[:, NS + ch * 512:NS + (ch + 1) * 512]
                else:
                    def src0(ch, _l=l):
                        return cur_x(slice(ch * 512, (ch + 1) * 512))

                    def src1(ch, _l=l):
                        return xob[_l - 1][:, ch * 512:(ch + 1) * 512]

                # own half (no collective dependency)
                do_half(l, 0, src0)
                # U = exp(.99 f1) broadcast: PE ones-outer bcast + ACT exp
                psF1 = bigp.tile([P, NS], F32, name=f"f1b{l}", tag="big")
                for ch in range(nch):
                    sl = slice(ch * 512, (ch + 1) * 512)
                    nc.tensor.matmul(psF1[:, sl], onesrow_r[:],
                                     f1row[0:1, sl], start=True, stop=True)
                nc.scalar.activation(U[:], psF1[:], AF.Exp, scale=0.99)

                psAT = attp.tile([P, NS], F32, name=f"psAT{l}", tag="att")
                psS = srp.tile([1, NS], F32, name=f"psS{l}", tag="srow")
                if l == 0:
                    do_half(l, 1, src1)
                    do_jtiles(l, psAT, psS, 0, nj)
                else:
                    # hide the AllReduce behind the own-half j-tiles
                    do_jtiles(l, psAT, psS, 0, njh)
                    nc.sync.dma_start(sumb[l - 1][:], ar_out[l - 1].ap())
                    nc.vector.tensor_tensor(xob[l - 1][:], sumb[l - 1][:],
                                            cur_x(slice(0, NS)),
                                            OP.subtract)
                    do_half(l, 1, src1)
                    do_jtiles(l, psAT, psS, njh, nj)

                # normalize + residual: x_new = psAT/S + x, chunked so the
                # next layer's h starts on chunk 0 early.
                # 1/S = exp(-ln(S)) on ACT (custom-DVE recip doesn't lower
                # on this toolchain; InstReciprocal costs ~6x elem rate)
                nc.scalar.activation(Rv_row[:], psS[:], AF.Ln)
                nc.scalar.activation(rrow[:], Rv_row[:], AF.Exp,
                                     scale=-1.0)
                psRv = bigp.tile([P, NS], F32, name=f"rvb{l}", tag="big")
                xb_new = xtp.tile([P, NS], BF16, name=f"xb{l + 1}",
                                  tag="xb")
                tmpb = qp.tile([P, NS], BF16, name=f"tmpb{l}", tag="q2")
                for ch in range(nch):
                    sl = slice(ch * 512, (ch + 1) * 512)
                    nc.tensor.matmul(psRv[:, sl], onesrow_r[:],
                                     rrow[0:1, sl], start=True, stop=True)
                    nc.scalar.activation(Rvb[:, sl], psRv[:, sl], AF.Copy)
                    nc.vector.tensor_tensor(tmpb[:, sl], psAT[:, sl],
                                            Rvb[:, sl], OP.mult)
                    nc.vector.tensor_tensor(xb_new[:, sl], tmpb[:, sl],
                                            cur_x(sl), OP.add)
                xb_cur[0] = (xb_new, 0)

                if not last:
                    nc.gpsimd.dma_start(ar_in[l].ap(), xb_new[:])
                    nc.gpsimd.collective_compute(
                        "AllReduce", OP.add, replica_groups=pair_groups,
                        ins=[ar_in[l].ap()], outs=[ar_out[l].ap()])

            # ---- final linear: outT = relu(Wt @ x + bt), bf16, transposed
            for g in range(nH):
                for ch in range(nch):
                    sl = slice(ch * 512, (ch + 1) * 512)
                    ps = hp.tile([P, 512], F32, name=f"ops{g}_{ch}",
                                 tag="hps")
                    nc.tensor.matmul(ps[:], WtTt[:, g * P:(g + 1) * P],
                                     cur_x(sl), start=True, stop=True)
                    oc = ocp.tile([P, 512], BF16, name=f"oc{g}_{ch}",
                                  tag="oc")
                    nc.scalar.activation(oc[:], ps[:], AF.Relu,
                                         bias=btpt[:, g:g + 1])
                    nc.sync.dma_start(
                        out_ext.ap()[g * P:(g + 1) * P, sl], oc[:])

    if legalize:
        _legalize_waits(nc)
    return nc


def make_in_maps(x, adj, Ws, bs, avs, Wt, bt, num_cores, NS):
    """Per-core input dicts. Core c -> (graph c//2, row-half c%2).
    Per-core the atom (column) axis is permuted to [own half | other];
    adjacency is sent pre-transposed ([j, i]) in bf16."""
    B, N, D = x.shape
    H = Wt.shape[0]
    nH = H // P
    x = np.asarray(x, np.float32)
    adj = np.asarray(adj)
    shared = {"WtT": np.ascontiguousarray(
                  np.asarray(Wt, np.float32).T.astype(_BF16NP)),
              "btp": np.ascontiguousarray(
                  np.asarray(bt, np.float32).reshape(nH, P).T),
              "arw_in": np.zeros((1, 1), np.float32)}
    for l, (W, b, a) in enumerate(zip(Ws, bs, avs)):
        shared[f"WT{l}"] = np.ascontiguousarray(
            np.asarray(W, np.float32).T.astype(_BF16NP))
        shared[f"bv{l}"] = np.ascontiguousarray(
            np.asarray(b, np.float32).reshape(D, 1))
        shared[f"av{l}"] = np.ascontiguousarray(
            np.stack([np.asarray(a, np.float32)[:D, 0],
                      np.asarray(a, np.float32)[D:, 0]],
                     axis=1).astype(_BF16NP))
    in_maps = []
    for c in range(num_cores):
        b, s = c // 2, c % 2
        perm = np.concatenate([np.arange(s * NS, (s + 1) * NS),
                               np.arange((1 - s) * NS, (2 - s) * NS)])
        m = dict(shared)
        m["xTb"] = np.ascontiguousarray(x[b][perm].T.astype(_BF16NP))
        m["adjT_s"] = np.ascontiguousarray(
            adj[b, s * NS:(s + 1) * NS][:, perm].T.astype(_BF16NP))
        in_maps.append(m)
    return in_maps


_NC_CACHE = {}


def kernel(x, adj, W0, b0, W1, b1, W2, b2, a0, a1, a2, Wt, bt):
    B, N, D = 4, 2048, 128
    H = 256
    NUM_CORES = 8
    NS = N // 2
    pair_groups = [[2 * i, 2 * i + 1] for i in range(NUM_CORES // 2)]

    key = (N, NS, D, H, NUM_CORES)
    if key not in _NC_CACHE:
        _NC_CACHE[key] = build_gat_nc(N, NS, D, H, NUM_CORES, pair_groups)
    nc = _NC_CACHE[key]

    in_maps = make_in_maps(np.asarray(x), np.asarray(adj),
                           [W0, W1, W2], [b0, b1, b2], [a0, a1, a2],
                           np.asarray(Wt), np.asarray(bt), NUM_CORES, NS)
    res = run_bass_kernel_spmd(nc, in_maps, list(range(NUM_CORES))).results
    out = np.empty((B, N, H), np.float32)
    for c in range(NUM_CORES):
        b, s = c // 2, c % 2
        out[b, s * NS:(s + 1) * NS, :] = \
            res[c]["outT_s"].astype(np.float32).T
    return out
